# revision 4
# baseline (speedup 1.0000x reference)
"""LightGCN (3-layer) + BPR loss on 8 Trainium2 NeuronCores.

Strategy (graph-parallel over edge destinations):
  - Remap nodes so core c owns a contiguous padded slab of 20480 node slots
    (160 windows x 128); sort edges by destination and shard by dst slab.
  - Per layer, per core: gather x[src] rows (256B) with dma_gather (int16
    local indices, node table split into 5 x 32768-row chunks), build a
    scaled one-hot S[e, dst_local] = val_e with one DVE tensor_scalar
    (iota, is_equal, mult), and segment-sum via TensorE matmuls
    accumulating in PSUM over a 16-window superblock. Flush adds into an
    SBUF-resident acc and writes the slab; AllGather replicates the new
    node table for the next layer's gathers.
  - BPR tail is data-parallel over the 4096 batch (512/core): indirect
    row gathers + DVE dot products + ScalarE softplus + a ones-matmul
    partition reduction. Host sums the 8 partial (loss, reg) pairs.
"""

import sys

sys.path.insert(0, "/opt/trn_rl_repo")

import numpy as np

P = 128
D = 64
CORES = 8
N_USERS = 100000
N_ITEMS = 50000
N = N_USERS + N_ITEMS  # 150000
SLAB_REAL = N // CORES  # 18750
WPC = 160  # windows per core (147 real + 13 dead, for 10 uniform superblocks)
SLABP = WPC * P  # 20480 padded node slots per core
NP_TOTAL = CORES * SLABP  # 163840
CHUNK = 32768  # dma_gather int16 index reach
NCHUNK = NP_TOTAL // CHUNK  # 5
SBW = 16  # windows per superblock
NSB = WPC // SBW  # 10
BATCH = 4096
BSH = BATCH // CORES  # 512 batch rows per core
BT = BSH // P  # 4 batch tiles per core


def _remap(n):
    """global node id -> padded id (core-contiguous slabs)"""
    return (n // SLAB_REAL) * SLABP + (n % SLAB_REAL)


def preprocess(user_emb, item_emb, edge_vals, edge_src, edge_dst, users, pos, neg):
    """Host-side: build the padded node table, per-core edge streams, static
    tile maps shared by all cores, and BPR index tiles."""
    src_m = _remap(edge_src.astype(np.int64))
    dst_m = _remap(edge_dst.astype(np.int64))
    val = edge_vals.astype(np.float32)

    x0 = np.zeros((NP_TOTAL, D), dtype=np.float32)
    emb = np.concatenate([user_emb, item_emb], axis=0).astype(np.float32)
    x0[_remap(np.arange(N))] = emb

    core = dst_m // SLABP
    dst_local = dst_m - core * SLABP
    win = dst_local >> 7  # window within core
    chunk = src_m // CHUNK
    sb = win // SBW
    wr = win - sb * SBW  # window within superblock
    wkey = np.where(chunk % 2 == 0, wr, SBW - 1 - wr)  # serpentine

    # per (core, window, chunk) counts -> static quotas (max over cores)
    flat = (core * WPC + win) * NCHUNK + chunk
    counts = np.bincount(flat, minlength=CORES * WPC * NCHUNK).reshape(
        CORES, WPC, NCHUNK
    )
    Q = counts.max(axis=0)  # [WPC, NCHUNK]

    # static slot layout: superblock -> chunk -> serpentine windows
    # group sizes (pre-pad) per (sb, chunk)
    grp_sizes = np.zeros((NSB, NCHUNK), dtype=np.int64)
    for s in range(NSB):
        for c in range(NCHUNK):
            grp_sizes[s, c] = Q[s * SBW : (s + 1) * SBW, c].sum()
    grp_pad = ((grp_sizes + P - 1) // P) * P  # 128-aligned groups
    sb_sizes = grp_pad.sum(axis=1)  # slots per superblock
    sb_starts = np.concatenate([[0], np.cumsum(sb_sizes)])
    TOT = int(sb_starts[-1])
    NTILES = TOT // P

    # run starts per (window, chunk) in slot space + per-slot window map
    run_start = np.zeros((WPC, NCHUNK), dtype=np.int64)
    W_slot = np.zeros(TOT, dtype=np.int32)
    grp_start = np.zeros((NSB, NCHUNK), dtype=np.int64)
    for s in range(NSB):
        off = sb_starts[s]
        for c in range(NCHUNK):
            grp_start[s, c] = off
            ws = range(s * SBW, (s + 1) * SBW)
            order = list(ws) if c % 2 == 0 else list(ws)[::-1]
            last_w = order[0]
            for w in order:
                run_start[w, c] = off
                W_slot[off : off + Q[w, c]] = w
                if Q[w, c] > 0:
                    last_w = w
                off += Q[w, c]
            # group-end pad slots: last window that actually received slots
            pad_end = grp_start[s, c] + grp_pad[s, c]
            if off < pad_end:
                W_slot[off:pad_end] = last_w
            off = pad_end

    # tile maps (static, same all cores)
    tw = W_slot.reshape(NTILES, P)
    tile_minw = tw.min(axis=1)
    tile_maxw = tw.max(axis=1)
    assert (tile_maxw - tile_minw <= 1).all(), "tile spans >2 windows"
    # which tile is each window's first/last segment (slot order)
    first_tile = np.full(WPC, -1, dtype=np.int64)
    last_tile = np.full(WPC, -1, dtype=np.int64)
    for t in range(NTILES):
        for w in range(tile_minw[t], tile_maxw[t] + 1):
            if first_tile[w] < 0:
                first_tile[w] = t
            last_tile[w] = t

    # fill per-core streams
    dst_rel_default = (W_slot.astype(np.int64) * P) - tile_minw.repeat(P) * P
    idx_all = np.zeros((CORES, TOT), dtype=np.int16)  # chunk-local src idx
    val_all = np.zeros((CORES, TOT), dtype=np.float32)
    dstr_all = np.broadcast_to(
        dst_rel_default.astype(np.float32), (CORES, TOT)
    ).copy()

    # slot assignment: order edges by (core, run), cumcount within run
    run_id = (core * WPC + win) * NCHUNK + chunk
    order = np.lexsort((src_m, run_id))
    rid_s = run_id[order]
    # position within run (rid_s is sorted)
    starts = np.concatenate([[0], np.flatnonzero(rid_s[1:] != rid_s[:-1]) + 1])
    lens = np.diff(np.concatenate([starts, [len(rid_s)]]))
    run_pos = np.arange(len(rid_s)) - np.repeat(starts, lens)
    slot = run_start[win[order], chunk[order]] + run_pos
    c_o = core[order]
    idx_all[c_o, slot] = (src_m[order] - chunk[order] * CHUNK).astype(np.int16)
    val_all[c_o, slot] = val[order]
    dstr = dst_local[order] - tile_minw[slot // P].astype(np.int64) * P
    dstr_all[c_o, slot] = dstr.astype(np.float32)

    # wrap idxs per gather group: [TOT] -> [128, TOT//16] (16-wrap, replicated)
    idx_wrapped = np.zeros((CORES, P, TOT // 16), dtype=np.int16)
    for s in range(NSB):
        for c in range(NCHUNK):
            g0, g1 = grp_start[s, c], grp_start[s, c] + grp_pad[s, c]
            blk = idx_all[:, g0:g1].reshape(CORES, -1, 16).transpose(0, 2, 1)
            idx_wrapped[:, :16, g0 // 16 : g1 // 16] = blk
    idx_wrapped[:, 16:, :] = np.tile(idx_wrapped[:, :16, :], (1, 7, 1))

    # [TOT] -> [128, NTILES] tile-major for dst/val
    val_t = val_all.reshape(CORES, NTILES, P).transpose(0, 2, 1).copy()
    dst_t = dstr_all.reshape(CORES, NTILES, P).transpose(0, 2, 1).copy()

    # per-core x0 slab (for acc init)
    x0_slab = x0.reshape(CORES, SLABP, D)

    # BPR per-core index tiles [128, BT] int32 (padded-id row offsets)
    def btile(ids):
        return ids.reshape(BT, P).T.astype(np.int32).copy()

    u_g = _remap(users.astype(np.int64))
    p_g = _remap(N_USERS + pos.astype(np.int64))
    n_g = _remap(N_USERS + neg.astype(np.int64))
    bpr = np.stack([u_g, p_g, n_g]).reshape(3, CORES, BSH)  # [3, CORES, 512]

    static = dict(
        TOT=TOT,
        NTILES=NTILES,
        grp_start=grp_start,
        grp_pad=grp_pad,
        sb_starts=sb_starts,
        tile_minw=tile_minw,
        tile_maxw=tile_maxw,
        first_tile=first_tile,
        last_tile=last_tile,
        W_slot=W_slot,
    )
    percore = []
    for c in range(CORES):
        percore.append(
            dict(
                idx=idx_wrapped[c],
                val=val_t[c],
                dst=dst_t[c],
                x0_slab=x0_slab[c].copy(),
                u_idx=btile(bpr[0, c]),
                p_idx=btile(bpr[1, c]),
                n_idx=btile(bpr[2, c]),
            )
        )
    return x0, static, percore


def numpy_mirror(x0, static, percore):
    """Debug: simulate the device computation with numpy. Returns the new
    full node table after one propagation layer (all cores)."""
    TOT, NTILES = static["TOT"], static["NTILES"]
    tile_minw = static["tile_minw"]
    out = np.zeros((NP_TOTAL, D), dtype=np.float32)
    for c in range(CORES):
        pc = percore[c]
        # unwrap idx
        idx = np.zeros(TOT, dtype=np.int64)
        for s in range(NSB):
            for ch in range(NCHUNK):
                g0 = static["grp_start"][s, ch]
                g1 = g0 + static["grp_pad"][s, ch]
                blk = pc["idx"][:16, g0 // 16 : g1 // 16]
                idx[g0:g1] = blk.T.reshape(-1).astype(np.int64) + ch * CHUNK
        gathered = x0[idx]  # [TOT, D]
        val = pc["val"].T.reshape(-1)
        dstr = pc["dst"].T.reshape(-1).astype(np.int64)
        node = dstr + tile_minw.repeat(P) * P + c * SLABP
        np.add.at(out, node, gathered * val[:, None])
    return out


def build_program(static, nsb_limit=NSB, nlayers=3, do_ag=True, do_bpr=True,
                  do_gather=True, do_s=True, do_mm=True, do_flush=True,
                  do_idxload=True):
    import concourse.bacc as bacc
    import concourse.bass as bass
    import concourse.mybir as mybir
    import concourse.tile as tile

    TOT, NTILES = static["TOT"], static["NTILES"]
    grp_start, grp_pad = static["grp_start"], static["grp_pad"]
    sb_starts = static["sb_starts"]
    tile_minw, tile_maxw = static["tile_minw"], static["tile_maxw"]
    first_tile, last_tile = static["first_tile"], static["last_tile"]

    f32 = mybir.dt.float32
    nc = bacc.Bacc(
        "TRN2",
        target_bir_lowering=False,
        debug=False,
        num_devices=CORES,
        num_swdge_queues=4,
    )

    x0_ext = nc.dram_tensor("x0", [NP_TOTAL, D], f32, kind="ExternalInput")
    x0_slab = nc.dram_tensor("x0_slab", [SLABP, D], f32, kind="ExternalInput")
    idx_in = nc.dram_tensor("idx", [P, TOT // 16], mybir.dt.int16, kind="ExternalInput")
    val_in = nc.dram_tensor("val", [P, NTILES], f32, kind="ExternalInput")
    dst_in = nc.dram_tensor("dst", [P, NTILES], f32, kind="ExternalInput")
    iota_in = nc.dram_tensor("iota", [P, 2 * P], f32, kind="ExternalInput")
    ones_in = nc.dram_tensor("ones", [P, 1], f32, kind="ExternalInput")
    u_in = nc.dram_tensor("u_idx", [P, BT], mybir.dt.int32, kind="ExternalInput")
    p_in = nc.dram_tensor("p_idx", [P, BT], mybir.dt.int32, kind="ExternalInput")
    n_in = nc.dram_tensor("n_idx", [P, BT], mybir.dt.int32, kind="ExternalInput")
    out_sc = nc.dram_tensor("out_sc", [2, 1], f32, kind="ExternalOutput")

    with tile.TileContext(nc) as tc:
        with (
            tc.tile_pool(name="const", bufs=1) as cpool,
            tc.tile_pool(name="acc", bufs=1) as apool,
            tc.tile_pool(name="stream", bufs=2) as stpool,
            tc.tile_pool(name="idxp", bufs=8) as idxpool,
            tc.tile_pool(name="gb", bufs=4) as gpool,
            tc.tile_pool(name="s", bufs=12) as spool,
            tc.tile_pool(name="fl", bufs=2) as fpool,
            tc.tile_pool(name="psum", bufs=2, space="PSUM") as ppool,
            tc.tile_pool(name="bsum", bufs=1, space="PSUM") as bppool,
            tc.tile_pool(name="bpr", bufs=1) as bpool,
            tc.tile_pool(name="dram", bufs=1, space="DRAM") as dpool,
        ):
            iota_sb = cpool.tile([P, 2 * P], f32)
            nc.sync.dma_start(out=iota_sb[:], in_=iota_in[:])
            ones_sb = cpool.tile([P, 1], f32)
            nc.sync.dma_start(out=ones_sb[:], in_=ones_in[:])

            # SBUF-resident accumulator [128, WPC*D], window w at cols w*D
            acc_sb = apool.tile([P, WPC * D], f32)
            nc.sync.dma_start(
                out=acc_sb[:].rearrange("p (w d) -> p w d", d=D),
                in_=x0_slab[:].rearrange("(w p) d -> p w d", p=P),
            )

            # DRAM internals
            slab_dram = [dpool.tile([SLABP, D], f32, name=f"slab{l}") for l in range(3)]
            xg = [dpool.tile([NP_TOTAL, D], f32, name=f"xg{l}") for l in range(2)]
            acc_slab_dram = dpool.tile([SLABP, D], f32)
            acc_full = dpool.tile([NP_TOTAL, D], f32)

            gsrc = [x0_ext, xg[0]] + [xg[1]] * max(1, nlayers - 2)

            max_grp = int(grp_pad.max())
            gq = 0
            idx_fixed = None
            if not do_idxload:
                idx_fixed = cpool.tile([P, max_grp // 16], mybir.dt.int16)
                nc.sync.dma_start(
                    out=idx_fixed[:], in_=idx_in[:, : max_grp // 16]
                )
            for layer in range(nlayers):
                src_t = gsrc[layer]
                for s in range(nsb_limit):
                    t0 = int(sb_starts[s]) // P
                    t1 = int(sb_starts[s + 1]) // P
                    # stream tiles for this superblock
                    ntile_s = t1 - t0
                    val_sb = stpool.tile([P, ntile_s], f32, tag="val")
                    nc.sync.dma_start(out=val_sb[:], in_=val_in[:, t0:t1])
                    dst_sb = stpool.tile([P, ntile_s], f32, tag="dst")
                    nc.sync.dma_start(out=dst_sb[:], in_=dst_in[:, t0:t1])

                    psum = ppool.tile([P, SBW * D], f32, space="PSUM")
                    for ch in range(NCHUNK):
                        g0 = int(grp_start[s, ch])
                        gl = int(grp_pad[s, ch])
                        if gl == 0:
                            continue
                        if do_idxload:
                            idx_sb = idxpool.tile(
                                [P, max_grp // 16], mybir.dt.int16, tag="idx"
                            )
                            nc.sync.dma_start(
                                out=idx_sb[:, : gl // 16],
                                in_=idx_in[:, g0 // 16 : (g0 + gl) // 16],
                            )
                        else:
                            idx_sb = idx_fixed
                        gbuf = gpool.tile([P, (max_grp // P) * D], f32, tag="gbuf")
                        if do_gather:
                          nc.gpsimd.dma_gather(
                            gbuf[:, : (gl // P) * D].rearrange(
                                "p (t d) -> p t d", d=D
                            ),
                            src_t[ch * CHUNK : (ch + 1) * CHUNK, :],
                            idx_sb[:, : gl // 16],
                            gl,
                            gl,
                            D,
                            single_packet=False,
                            queue_num=gq % 4,
                          )
                        gq += 1
                        for tt in range(g0 // P, (g0 + gl) // P):
                            trel = tt - t0
                            gt = tt - g0 // P
                            minw, maxw = int(tile_minw[tt]), int(tile_maxw[tt])
                            nwin = maxw - minw + 1
                            s_t = spool.tile([P, 2 * P], f32, tag="s_t")
                            if do_s:
                              nc.any.tensor_scalar(
                                out=s_t[:, : nwin * P],
                                in0=iota_sb[:, : nwin * P],
                                scalar1=dst_sb[:, trel : trel + 1],
                                scalar2=val_sb[:, trel : trel + 1],
                                op0=mybir.AluOpType.is_equal,
                                op1=mybir.AluOpType.mult,
                              )
                            for k in range(nwin):
                                w = minw + k
                                wr = w - s * SBW
                                if do_mm:
                                    nc.tensor.matmul(
                                        out=psum[:, wr * D : (wr + 1) * D],
                                        lhsT=s_t[:, k * P : (k + 1) * P],
                                        rhs=gbuf[:, gt * D : (gt + 1) * D],
                                        start=(first_tile[w] == tt),
                                        stop=(last_tile[w] == tt),
                                    )

                    # flush superblock: psum -> sbuf, acc +=, slab write
                    if not do_flush:
                        continue
                    flush = fpool.tile([P, SBW * D], f32, tag="flush")
                    nc.scalar.copy(out=flush[:], in_=psum[:])
                    nc.vector.tensor_tensor(
                        out=acc_sb[:, s * SBW * D : (s + 1) * SBW * D],
                        in0=acc_sb[:, s * SBW * D : (s + 1) * SBW * D],
                        in1=flush[:],
                        op=mybir.AluOpType.add,
                    )
                    nc.sync.dma_start(
                        out=slab_dram[min(layer, 2)][
                            s * SBW * P : (s + 1) * SBW * P, :
                        ].rearrange("(w p) d -> p w d", p=P),
                        in_=flush[:].rearrange("p (w d) -> p w d", d=D),
                    )

                if layer < 2 and do_ag:
                    nc.gpsimd.collective_compute(
                        "AllGather",
                        mybir.AluOpType.bypass,
                        replica_groups=[list(range(CORES))],
                        ins=[slab_dram[layer].opt()],
                        outs=[xg[layer].opt()],
                    )

            # acc -> DRAM slab -> AllGather
            if do_ag:
              nc.sync.dma_start(
                out=acc_slab_dram[:].rearrange("(w p) d -> p w d", p=P),
                in_=acc_sb[:].rearrange("p (w d) -> p w d", d=D),
              )
              nc.gpsimd.collective_compute(
                "AllGather",
                mybir.AluOpType.bypass,
                replica_groups=[list(range(CORES))],
                ins=[acc_slab_dram.opt()],
                outs=[acc_full.opt()],
              )

            # ---- BPR tail ----
            if not do_bpr:
                zt = bpool.tile([2, 1], f32, name='zt')
                nc.vector.memset(zt[:], 0.0)
                nc.sync.dma_start(out=out_sc[:], in_=zt[:])
            else:
              bidx = {"u": u_in, "p": p_in, "n": n_in}
              bsb = {}
              for k, t_in in bidx.items():
                  tl = bpool.tile([P, BT], mybir.dt.int32, name=f"bi_{k}")
                  nc.sync.dma_start(out=tl[:], in_=t_in[:])
                  bsb[k] = tl

              def gather_rows(table, idx_tile, name):
                  dst = bpool.tile([P, BT * D], f32, name=f"g_{name}")
                  for j in range(BT):
                      nc.gpsimd.indirect_dma_start(
                          out=dst[:, j * D : (j + 1) * D],
                          out_offset=None,
                          in_=table[:],
                          in_offset=bass.IndirectOffsetOnAxis(
                              ap=idx_tile[:, j : j + 1], axis=0
                          ),
                      )
                  return dst

              gu = gather_rows(acc_full, bsb["u"], "u")
              gp = gather_rows(acc_full, bsb["p"], "p")
              gn = gather_rows(acc_full, bsb["n"], "n")
              g0u = gather_rows(x0_ext, bsb["u"], "u0")
              g0p = gather_rows(x0_ext, bsb["p"], "p0")
              g0n = gather_rows(x0_ext, bsb["n"], "n0")

              # lightgcn output = acc / 4
              # scores: sum over D of (gu/4)*(gp/4) = dot(gu,gp)/16
              tmp = bpool.tile([P, BT * D], f32, name="tmp")
              ps = bpool.tile([P, BT], f32, name="ps")
              ns_ = bpool.tile([P, BT], f32, name="ns")
              nc.vector.tensor_tensor(
                  out=tmp[:], in0=gu[:], in1=gp[:], op=mybir.AluOpType.mult
              )
              nc.vector.tensor_reduce(
                  out=ps[:],
                  in_=tmp[:].rearrange("p (t d) -> p t d", d=D),
                  axis=mybir.AxisListType.X,
                  op=mybir.AluOpType.add,
              )
              nc.vector.tensor_tensor(
                  out=tmp[:], in0=gu[:], in1=gn[:], op=mybir.AluOpType.mult
              )
              nc.vector.tensor_reduce(
                  out=ns_[:],
                  in_=tmp[:].rearrange("p (t d) -> p t d", d=D),
                  axis=mybir.AxisListType.X,
                  op=mybir.AluOpType.add,
              )
              # diff = (ns - ps)/16 ; softplus ; sum over batch tiles
              diff = bpool.tile([P, BT], f32, name="diff")
              nc.vector.tensor_tensor(
                  out=diff[:], in0=ns_[:], in1=ps[:], op=mybir.AluOpType.subtract
              )
              # softplus(diff/16) = ln(1 + exp(diff/16)); scores are tiny so
              # exp cannot overflow
              sp = bpool.tile([P, BT], f32, name="sp")
              nc.scalar.activation(
                  out=sp[:],
                  in_=diff[:],
                  func=mybir.ActivationFunctionType.Exp,
                  scale=1.0 / 16.0,
              )
              nc.vector.tensor_scalar(
                  out=sp[:],
                  in0=sp[:],
                  scalar1=1.0,
                  scalar2=None,
                  op0=mybir.AluOpType.add,
              )
              nc.scalar.activation(
                  out=sp[:], in_=sp[:], func=mybir.ActivationFunctionType.Ln
              )
              # reg: sum of squares of u0,p0,n0
              sq = bpool.tile([P, BT], f32, name="sq")
              red2 = bpool.tile([P, 2], f32, name="red2")
              nc.vector.tensor_reduce(
                  out=red2[:, 0:1],
                  in_=sp[:],
                  axis=mybir.AxisListType.X,
                  op=mybir.AluOpType.add,
              )
              for i, g in enumerate([g0u, g0p, g0n]):
                  nc.vector.tensor_tensor(
                      out=tmp[:], in0=g[:], in1=g[:], op=mybir.AluOpType.mult
                  )
                  nc.vector.tensor_reduce(
                      out=sq[:],
                      in_=tmp[:].rearrange("p (t d) -> p t d", d=D),
                      axis=mybir.AxisListType.X,
                      op=mybir.AluOpType.add,
                  )
                  if i == 0:
                      nc.vector.tensor_reduce(
                          out=red2[:, 1:2],
                          in_=sq[:],
                          axis=mybir.AxisListType.X,
                          op=mybir.AluOpType.add,
                      )
                  else:
                      sq1 = bpool.tile([P, 1], f32, name="sq1")
                      nc.vector.tensor_reduce(
                          out=sq1[:],
                          in_=sq[:],
                          axis=mybir.AxisListType.X,
                          op=mybir.AluOpType.add,
                      )
                      nc.vector.tensor_tensor(
                          out=red2[:, 1:2],
                          in0=red2[:, 1:2],
                          in1=sq1[:],
                          op=mybir.AluOpType.add,
                      )
              # partition reduce via ones matmul: out[2,1] = red2.T @ ones
              bp_ps = bppool.tile([2, 1], f32, space="PSUM")
              nc.tensor.matmul(
                  out=bp_ps[:], lhsT=red2[:], rhs=ones_sb[:], start=True, stop=True
              )
              sc = bpool.tile([2, 1], f32, name="sc")
              nc.vector.tensor_copy(out=sc[:], in_=bp_ps[:])
              nc.sync.dma_start(out=out_sc[:], in_=sc[:])

    nc.compile()
    return nc


_LAST_EXEC_NS = None
_LAST_RUN_SECONDS = None
_LAST_RES = None


def kernel(user_emb, item_emb, edge_vals, edge_src, edge_dst, users, pos, neg):
    global _LAST_EXEC_NS, _LAST_RUN_SECONDS, _LAST_RES
    import time as _time

    from concourse.bass_utils import run_bass_kernel_spmd

    x0, static, percore = preprocess(
        user_emb, item_emb, edge_vals, edge_src, edge_dst, users, pos, neg
    )
    nc = build_program(static)

    iota = np.broadcast_to(np.arange(2 * P, dtype=np.float32), (P, 2 * P)).copy()
    ones = np.ones((P, 1), dtype=np.float32)
    in_maps = []
    for c in range(CORES):
        pc = percore[c]
        in_maps.append(
            {
                "x0": x0,
                "x0_slab": pc["x0_slab"],
                "idx": pc["idx"],
                "val": pc["val"],
                "dst": pc["dst"],
                "iota": iota,
                "ones": ones,
                "u_idx": pc["u_idx"],
                "p_idx": pc["p_idx"],
                "n_idx": pc["n_idx"],
            }
        )

    _t0 = _time.time()
    res = run_bass_kernel_spmd(nc, in_maps, core_ids=list(range(CORES)))
    _LAST_RUN_SECONDS = _time.time() - _t0
    _LAST_EXEC_NS = res.exec_time_ns
    _LAST_RES = res
    loss = np.float32(0.0)
    reg_raw = np.float32(0.0)
    for c in range(CORES):
        sc = res.results[c]["out_sc"]
        loss += sc[0, 0]
        reg_raw += sc[1, 0]
    reg_loss = np.float32(0.5) * reg_raw / np.float32(BATCH)
    return np.float32(loss), np.float32(reg_loss)



# revision 18
# speedup vs baseline: 1.1623x; 1.1623x over previous
"""LightGCN (3-layer) + BPR loss on 8 Trainium2 NeuronCores.

Strategy (graph-parallel over edge destinations):
  - Remap nodes so core c owns a contiguous padded slab of 20480 node slots
    (160 windows x 128); sort edges by destination and shard by dst slab.
  - Per layer, per core: gather x[src] rows (256B) with dma_gather (int16
    local indices, node table split into 5 x 32768-row chunks), build a
    scaled one-hot S[e, dst_local] = val_e with one DVE tensor_scalar
    (iota, is_equal, mult), and segment-sum via TensorE matmuls
    accumulating in PSUM over a 16-window superblock. Flush adds into an
    SBUF-resident acc and writes the slab; AllGather replicates the new
    node table for the next layer's gathers.
  - BPR tail is data-parallel over the 4096 batch (512/core): indirect
    row gathers + DVE dot products + ScalarE softplus + a ones-matmul
    partition reduction. Host sums the 8 partial (loss, reg) pairs.
"""

import sys

sys.path.insert(0, "/opt/trn_rl_repo")

import numpy as np

P = 128
D = 64
CORES = 8
N_USERS = 100000
N_ITEMS = 50000
N = N_USERS + N_ITEMS  # 150000
SLAB_REAL = N // CORES  # 18750
WPC = 160  # windows per core (147 real + 13 dead, for 10 uniform superblocks)
SLABP = WPC * P  # 20480 padded node slots per core
NP_TOTAL = CORES * SLABP  # 163840
CHUNK = 32768  # dma_gather int16 index reach
NCHUNK = NP_TOTAL // CHUNK  # 5
SBW = 16  # windows per superblock
NSB = WPC // SBW  # 10
BATCH = 4096
BSH = BATCH // CORES  # 512 batch rows per core
BT = BSH // P  # 4 batch tiles per core


def _remap(n):
    """global node id -> padded id (core-contiguous slabs)"""
    return (n // SLAB_REAL) * SLABP + (n % SLAB_REAL)


def preprocess(user_emb, item_emb, edge_vals, edge_src, edge_dst, users, pos, neg):
    """Host-side: build the padded node table, per-core edge streams, static
    tile maps shared by all cores, and BPR index tiles."""
    src_m = _remap(edge_src.astype(np.int64))
    dst_m = _remap(edge_dst.astype(np.int64))
    val = edge_vals.astype(np.float32)

    x0 = np.zeros((NP_TOTAL, D), dtype=np.float32)
    emb = np.concatenate([user_emb, item_emb], axis=0).astype(np.float32)
    x0[_remap(np.arange(N))] = emb
    # doubled-row bf16 table: row i = [x[i], zeros]; 256B rows keep dma_gather
    # legal while all compute runs bf16. cols 64:128 are never read.
    import ml_dtypes

    x0bf = np.zeros((NP_TOTAL, 2 * D), dtype=ml_dtypes.bfloat16)
    x0bf[:, :D] = x0.astype(ml_dtypes.bfloat16)

    core = dst_m // SLABP
    dst_local = dst_m - core * SLABP
    win = dst_local >> 7  # window within core
    chunk = src_m // CHUNK
    sb = win // SBW
    wr = win - sb * SBW  # window within superblock
    wkey = np.where(chunk % 2 == 0, wr, SBW - 1 - wr)  # serpentine

    # per (core, window, chunk) counts -> static quotas (max over cores)
    flat = (core * WPC + win) * NCHUNK + chunk
    counts = np.bincount(flat, minlength=CORES * WPC * NCHUNK).reshape(
        CORES, WPC, NCHUNK
    )
    Q = counts.max(axis=0)  # [WPC, NCHUNK]

    # static slot layout: superblock -> chunk -> serpentine windows
    # group sizes (pre-pad) per (sb, chunk)
    grp_sizes = np.zeros((NSB, NCHUNK), dtype=np.int64)
    for s in range(NSB):
        for c in range(NCHUNK):
            grp_sizes[s, c] = Q[s * SBW : (s + 1) * SBW, c].sum()
    grp_pad = ((grp_sizes + P - 1) // P) * P  # 128-aligned groups
    sb_sizes = grp_pad.sum(axis=1)  # slots per superblock
    sb_starts = np.concatenate([[0], np.cumsum(sb_sizes)])
    TOT = int(sb_starts[-1])
    NTILES = TOT // P

    # run starts per (window, chunk) in slot space + per-slot window map
    run_start = np.zeros((WPC, NCHUNK), dtype=np.int64)
    W_slot = np.zeros(TOT, dtype=np.int32)
    grp_start = np.zeros((NSB, NCHUNK), dtype=np.int64)
    for s in range(NSB):
        off = sb_starts[s]
        for c in range(NCHUNK):
            grp_start[s, c] = off
            ws = range(s * SBW, (s + 1) * SBW)
            order = list(ws) if c % 2 == 0 else list(ws)[::-1]
            last_w = order[0]
            for w in order:
                run_start[w, c] = off
                W_slot[off : off + Q[w, c]] = w
                if Q[w, c] > 0:
                    last_w = w
                off += Q[w, c]
            # group-end pad slots: last window that actually received slots
            pad_end = grp_start[s, c] + grp_pad[s, c]
            if off < pad_end:
                W_slot[off:pad_end] = last_w
            off = pad_end

    # tile maps (static, same all cores)
    tw = W_slot.reshape(NTILES, P)
    tile_minw = tw.min(axis=1)
    tile_maxw = tw.max(axis=1)
    assert (tile_maxw - tile_minw <= 1).all(), "tile spans >2 windows"
    # which tile is each window's first/last segment (slot order)
    first_tile = np.full(WPC, -1, dtype=np.int64)
    last_tile = np.full(WPC, -1, dtype=np.int64)
    for t in range(NTILES):
        for w in range(tile_minw[t], tile_maxw[t] + 1):
            if first_tile[w] < 0:
                first_tile[w] = t
            last_tile[w] = t

    # fill per-core streams
    dst_rel_default = (W_slot.astype(np.int64) * P) - tile_minw.repeat(P) * P
    idx_all = np.zeros((CORES, TOT), dtype=np.int16)  # chunk-local src idx
    val_all = np.zeros((CORES, TOT), dtype=np.float32)
    dstr_all = np.broadcast_to(
        dst_rel_default.astype(np.float32), (CORES, TOT)
    ).copy()

    # slot assignment: order edges by (core, run), cumcount within run
    run_id = (core * WPC + win) * NCHUNK + chunk
    order = np.lexsort((src_m, run_id))
    rid_s = run_id[order]
    # position within run (rid_s is sorted)
    starts = np.concatenate([[0], np.flatnonzero(rid_s[1:] != rid_s[:-1]) + 1])
    lens = np.diff(np.concatenate([starts, [len(rid_s)]]))
    run_pos = np.arange(len(rid_s)) - np.repeat(starts, lens)
    slot = run_start[win[order], chunk[order]] + run_pos
    c_o = core[order]
    idx_all[c_o, slot] = (src_m[order] - chunk[order] * CHUNK).astype(np.int16)
    val_all[c_o, slot] = val[order]
    dstr = dst_local[order] - tile_minw[slot // P].astype(np.int64) * P
    dstr_all[c_o, slot] = dstr.astype(np.float32)

    # wrap idxs per gather group: [TOT] -> [128, TOT//16] (16-wrap, replicated)
    idx_wrapped = np.zeros((CORES, P, TOT // 16), dtype=np.int16)
    for s in range(NSB):
        for c in range(NCHUNK):
            g0, g1 = grp_start[s, c], grp_start[s, c] + grp_pad[s, c]
            blk = idx_all[:, g0:g1].reshape(CORES, -1, 16).transpose(0, 2, 1)
            idx_wrapped[:, :16, g0 // 16 : g1 // 16] = blk
    idx_wrapped[:, 16:, :] = np.tile(idx_wrapped[:, :16, :], (1, 7, 1))

    # [TOT] -> [128, NTILES] tile-major for dst/val (f32: tensor_scalar
    # scalar operands must be f32)
    val_t = val_all.reshape(CORES, NTILES, P).transpose(0, 2, 1).copy()
    dst_t = dstr_all.reshape(CORES, NTILES, P).transpose(0, 2, 1).copy()

    # per-core x0 slab (for acc init)
    x0_slab = x0.reshape(CORES, SLABP, D)

    # BPR per-core index tiles [128, BT] int32 (padded-id row offsets)
    def btile(ids):
        return ids.reshape(BT, P).T.astype(np.int32).copy()

    u_g = _remap(users.astype(np.int64))
    p_g = _remap(N_USERS + pos.astype(np.int64))
    n_g = _remap(N_USERS + neg.astype(np.int64))
    bpr = np.stack([u_g, p_g, n_g]).reshape(3, CORES, BSH)  # [3, CORES, 512]

    static = dict(
        TOT=TOT,
        NTILES=NTILES,
        x0bf=x0bf,
        grp_start=grp_start,
        grp_pad=grp_pad,
        sb_starts=sb_starts,
        tile_minw=tile_minw,
        tile_maxw=tile_maxw,
        first_tile=first_tile,
        last_tile=last_tile,
        W_slot=W_slot,
    )
    percore = []
    for c in range(CORES):
        percore.append(
            dict(
                idx=idx_wrapped[c],
                val=val_t[c],
                dst=dst_t[c],
                x0_slab=x0_slab[c].copy(),
                u_idx=btile(bpr[0, c]),
                p_idx=btile(bpr[1, c]),
                n_idx=btile(bpr[2, c]),
            )
        )
    return x0, static, percore


def numpy_mirror(x0, static, percore):
    """Debug: simulate the device computation with numpy. Returns the new
    full node table after one propagation layer (all cores)."""
    TOT, NTILES = static["TOT"], static["NTILES"]
    tile_minw = static["tile_minw"]
    out = np.zeros((NP_TOTAL, D), dtype=np.float32)
    for c in range(CORES):
        pc = percore[c]
        # unwrap idx
        idx = np.zeros(TOT, dtype=np.int64)
        for s in range(NSB):
            for ch in range(NCHUNK):
                g0 = static["grp_start"][s, ch]
                g1 = g0 + static["grp_pad"][s, ch]
                blk = pc["idx"][:16, g0 // 16 : g1 // 16]
                idx[g0:g1] = blk.T.reshape(-1).astype(np.int64) + ch * CHUNK
        gathered = x0[idx]  # [TOT, D]
        val = pc["val"].T.reshape(-1)
        dstr = pc["dst"].T.reshape(-1).astype(np.int64)
        node = dstr + tile_minw.repeat(P) * P + c * SLABP
        np.add.at(out, node, gathered * val[:, None])
    return out


def build_program(static, nsb_limit=NSB, nlayers=3, do_ag=True, do_bpr=True,
                  do_gather=True, do_s=True, do_mm=True, do_flush=True,
                  do_idxload=True):
    import concourse.bacc as bacc
    import concourse.bass as bass
    import concourse.mybir as mybir
    import concourse.tile as tile

    TOT, NTILES = static["TOT"], static["NTILES"]
    grp_start, grp_pad = static["grp_start"], static["grp_pad"]
    sb_starts = static["sb_starts"]
    tile_minw, tile_maxw = static["tile_minw"], static["tile_maxw"]
    first_tile, last_tile = static["first_tile"], static["last_tile"]

    f32 = mybir.dt.float32
    bf16 = mybir.dt.bfloat16
    nc = bacc.Bacc(
        "TRN2",
        target_bir_lowering=False,
        debug=False,
        num_devices=CORES,
        num_swdge_queues=4,
    )

    x0_ext = nc.dram_tensor("x0", [NP_TOTAL, D], f32, kind="ExternalInput")
    x0bf_ext = nc.dram_tensor("x0bf", [NP_TOTAL, 2 * D], bf16, kind="ExternalInput")
    x0_slab = nc.dram_tensor("x0_slab", [SLABP, D], f32, kind="ExternalInput")
    idx_in = nc.dram_tensor("idx", [P, TOT // 16], mybir.dt.int16, kind="ExternalInput")
    val_in = nc.dram_tensor("val", [P, NTILES], f32, kind="ExternalInput")
    dst_in = nc.dram_tensor("dst", [P, NTILES], f32, kind="ExternalInput")
    iota_in = nc.dram_tensor("iota", [P, 2 * P], bf16, kind="ExternalInput")
    ones_in = nc.dram_tensor("ones", [P, 1], f32, kind="ExternalInput")
    u_in = nc.dram_tensor("u_idx", [P, BT], mybir.dt.int32, kind="ExternalInput")
    p_in = nc.dram_tensor("p_idx", [P, BT], mybir.dt.int32, kind="ExternalInput")
    n_in = nc.dram_tensor("n_idx", [P, BT], mybir.dt.int32, kind="ExternalInput")
    out_sc = nc.dram_tensor("out_sc", [2, 1], f32, kind="ExternalOutput")

    with tile.TileContext(nc) as tc:
        with (
            tc.tile_pool(name="const", bufs=1) as cpool,
            tc.tile_pool(name="acc", bufs=1) as apool,
            tc.tile_pool(name="stream", bufs=2) as stpool,
            tc.tile_pool(name="idxp", bufs=8) as idxpool,
            tc.tile_pool(name="gb", bufs=4) as gpool,
            tc.tile_pool(name="s", bufs=12) as spool,
            tc.tile_pool(name="fl", bufs=2) as fpool,
            tc.tile_pool(name="psum", bufs=2, space="PSUM") as ppool,
            tc.tile_pool(name="bsum", bufs=1, space="PSUM") as bppool,
            tc.tile_pool(name="bpr", bufs=1) as bpool,
            tc.tile_pool(name="dram", bufs=1, space="DRAM") as dpool,
        ):
            iota_sb = cpool.tile([P, 2 * P], bf16)
            nc.sync.dma_start(out=iota_sb[:], in_=iota_in[:])
            ones_sb = cpool.tile([P, 1], f32)
            nc.sync.dma_start(out=ones_sb[:], in_=ones_in[:])

            # SBUF-resident accumulator [128, WPC*D], window w at cols w*D
            acc_sb = apool.tile([P, WPC * D], f32)
            nc.sync.dma_start(
                out=acc_sb[:].rearrange("p (w d) -> p w d", d=D),
                in_=x0_slab[:].rearrange("(w p) d -> p w d", p=P),
            )

            # DRAM internals (node tables are doubled-row bf16; cols D:2D unread)
            slab_dram = [
                dpool.tile([SLABP, 2 * D], bf16, name=f"slab{l}") for l in range(3)
            ]
            xg = [dpool.tile([NP_TOTAL, 2 * D], bf16, name=f"xg{l}") for l in range(2)]
            acc_slab_dram = dpool.tile([SLABP, D], f32)
            acc_full = dpool.tile([NP_TOTAL, D], f32)

            gsrc = [x0bf_ext, xg[0]] + [xg[1]] * max(1, nlayers - 2)

            max_grp = int(grp_pad.max())
            gq = 0
            idx_fixed = None
            if not do_idxload:
                idx_fixed = cpool.tile([P, max_grp // 16], mybir.dt.int16)
                nc.sync.dma_start(
                    out=idx_fixed[:], in_=idx_in[:, : max_grp // 16]
                )
            for layer in range(nlayers):
                src_t = gsrc[layer]
                for s in range(nsb_limit):
                    t0 = int(sb_starts[s]) // P
                    t1 = int(sb_starts[s + 1]) // P
                    # stream tiles for this superblock
                    ntile_s = t1 - t0
                    val_sb = stpool.tile([P, ntile_s], f32, tag="val")
                    nc.sync.dma_start(out=val_sb[:], in_=val_in[:, t0:t1])
                    dst_sb = stpool.tile([P, ntile_s], f32, tag="dst")
                    nc.sync.dma_start(out=dst_sb[:], in_=dst_in[:, t0:t1])

                    psum = ppool.tile([P, SBW * D], f32, space="PSUM")
                    for ch in range(NCHUNK):
                        g0 = int(grp_start[s, ch])
                        gl = int(grp_pad[s, ch])
                        if gl == 0:
                            continue
                        if do_idxload:
                            idx_sb = idxpool.tile(
                                [P, max_grp // 16], mybir.dt.int16, tag="idx"
                            )
                            nc.sync.dma_start(
                                out=idx_sb[:, : gl // 16],
                                in_=idx_in[:, g0 // 16 : (g0 + gl) // 16],
                            )
                        else:
                            idx_sb = idx_fixed
                        gbuf = gpool.tile([P, (max_grp // P) * 2 * D], bf16, tag="gbuf")
                        if do_gather:
                          nc.gpsimd.dma_gather(
                            gbuf[:, : (gl // P) * 2 * D].rearrange(
                                "p (t d) -> p t d", d=2 * D
                            ),
                            src_t[ch * CHUNK : (ch + 1) * CHUNK, :],
                            idx_sb[:, : gl // 16],
                            gl,
                            gl,
                            2 * D,
                            single_packet=False,
                            queue_num=gq % 4,
                          )
                        gq += 1
                        for tt in range(g0 // P, (g0 + gl) // P):
                            trel = tt - t0
                            gt = tt - g0 // P
                            minw, maxw = int(tile_minw[tt]), int(tile_maxw[tt])
                            nwin = maxw - minw + 1
                            s_t = spool.tile([P, 2 * P], bf16, tag="s_t")
                            if do_s:
                              nc.any.tensor_scalar(
                                out=s_t[:, : nwin * P],
                                in0=iota_sb[:, : nwin * P],
                                scalar1=dst_sb[:, trel : trel + 1],
                                scalar2=val_sb[:, trel : trel + 1],
                                op0=mybir.AluOpType.is_equal,
                                op1=mybir.AluOpType.mult,
                              )
                            for k in range(nwin):
                                w = minw + k
                                wr = w - s * SBW
                                if do_mm:
                                    nc.tensor.matmul(
                                        out=psum[:, wr * D : (wr + 1) * D],
                                        lhsT=s_t[:, k * P : (k + 1) * P],
                                        rhs=gbuf[:, gt * 2 * D : gt * 2 * D + D],
                                        start=(first_tile[w] == tt),
                                        stop=(last_tile[w] == tt),
                                    )

                    # flush superblock: ACT casts psum -> bf16 slab tile, DVE
                    # adds psum into the f32 acc, slab x-half written to DRAM
                    if not do_flush:
                        continue
                    flush = fpool.tile([P, SBW * D], bf16, tag="flush")
                    nc.scalar.copy(out=flush[:], in_=psum[:])
                    nc.vector.tensor_tensor(
                        out=acc_sb[:, s * SBW * D : (s + 1) * SBW * D],
                        in0=acc_sb[:, s * SBW * D : (s + 1) * SBW * D],
                        in1=psum[:],
                        op=mybir.AluOpType.add,
                    )
                    nc.sync.dma_start(
                        out=slab_dram[min(layer, 2)][
                            s * SBW * P : (s + 1) * SBW * P, :D
                        ].rearrange("(w p) d -> p w d", p=P),
                        in_=flush[:].rearrange("p (w d) -> p w d", d=D),
                    )

                if layer < 2 and do_ag:
                    nc.gpsimd.collective_compute(
                        "AllGather",
                        mybir.AluOpType.bypass,
                        replica_groups=[list(range(CORES))],
                        ins=[slab_dram[layer].opt()],
                        outs=[xg[layer].opt()],
                    )

            # acc -> DRAM slab -> AllGather
            if do_ag:
              nc.sync.dma_start(
                out=acc_slab_dram[:].rearrange("(w p) d -> p w d", p=P),
                in_=acc_sb[:].rearrange("p (w d) -> p w d", d=D),
              )
              nc.gpsimd.collective_compute(
                "AllGather",
                mybir.AluOpType.bypass,
                replica_groups=[list(range(CORES))],
                ins=[acc_slab_dram.opt()],
                outs=[acc_full.opt()],
              )

            # ---- BPR tail ----
            if not do_bpr:
                zt = bpool.tile([2, 1], f32, name='zt')
                nc.vector.memset(zt[:], 0.0)
                nc.sync.dma_start(out=out_sc[:], in_=zt[:])
            else:
              bidx = {"u": u_in, "p": p_in, "n": n_in}
              bsb = {}
              for k, t_in in bidx.items():
                  tl = bpool.tile([P, BT], mybir.dt.int32, name=f"bi_{k}")
                  nc.sync.dma_start(out=tl[:], in_=t_in[:])
                  bsb[k] = tl

              def gather_rows(table, idx_tile, name):
                  dst = bpool.tile([P, BT * D], f32, name=f"g_{name}")
                  for j in range(BT):
                      nc.gpsimd.indirect_dma_start(
                          out=dst[:, j * D : (j + 1) * D],
                          out_offset=None,
                          in_=table[:],
                          in_offset=bass.IndirectOffsetOnAxis(
                              ap=idx_tile[:, j : j + 1], axis=0
                          ),
                      )
                  return dst

              gu = gather_rows(acc_full, bsb["u"], "u")
              gp = gather_rows(acc_full, bsb["p"], "p")
              gn = gather_rows(acc_full, bsb["n"], "n")
              g0u = gather_rows(x0_ext, bsb["u"], "u0")
              g0p = gather_rows(x0_ext, bsb["p"], "p0")
              g0n = gather_rows(x0_ext, bsb["n"], "n0")

              # lightgcn output = acc / 4
              # scores: sum over D of (gu/4)*(gp/4) = dot(gu,gp)/16
              tmp = bpool.tile([P, BT * D], f32, name="tmp")
              ps = bpool.tile([P, BT], f32, name="ps")
              ns_ = bpool.tile([P, BT], f32, name="ns")
              nc.vector.tensor_tensor(
                  out=tmp[:], in0=gu[:], in1=gp[:], op=mybir.AluOpType.mult
              )
              nc.vector.tensor_reduce(
                  out=ps[:],
                  in_=tmp[:].rearrange("p (t d) -> p t d", d=D),
                  axis=mybir.AxisListType.X,
                  op=mybir.AluOpType.add,
              )
              nc.vector.tensor_tensor(
                  out=tmp[:], in0=gu[:], in1=gn[:], op=mybir.AluOpType.mult
              )
              nc.vector.tensor_reduce(
                  out=ns_[:],
                  in_=tmp[:].rearrange("p (t d) -> p t d", d=D),
                  axis=mybir.AxisListType.X,
                  op=mybir.AluOpType.add,
              )
              # diff = (ns - ps)/16 ; softplus ; sum over batch tiles
              diff = bpool.tile([P, BT], f32, name="diff")
              nc.vector.tensor_tensor(
                  out=diff[:], in0=ns_[:], in1=ps[:], op=mybir.AluOpType.subtract
              )
              # softplus(diff/16) = ln(1 + exp(diff/16)); scores are tiny so
              # exp cannot overflow
              sp = bpool.tile([P, BT], f32, name="sp")
              nc.scalar.activation(
                  out=sp[:],
                  in_=diff[:],
                  func=mybir.ActivationFunctionType.Exp,
                  scale=1.0 / 16.0,
              )
              nc.vector.tensor_scalar(
                  out=sp[:],
                  in0=sp[:],
                  scalar1=1.0,
                  scalar2=None,
                  op0=mybir.AluOpType.add,
              )
              nc.scalar.activation(
                  out=sp[:], in_=sp[:], func=mybir.ActivationFunctionType.Ln
              )
              # reg: sum of squares of u0,p0,n0
              sq = bpool.tile([P, BT], f32, name="sq")
              red2 = bpool.tile([P, 2], f32, name="red2")
              nc.vector.tensor_reduce(
                  out=red2[:, 0:1],
                  in_=sp[:],
                  axis=mybir.AxisListType.X,
                  op=mybir.AluOpType.add,
              )
              for i, g in enumerate([g0u, g0p, g0n]):
                  nc.vector.tensor_tensor(
                      out=tmp[:], in0=g[:], in1=g[:], op=mybir.AluOpType.mult
                  )
                  nc.vector.tensor_reduce(
                      out=sq[:],
                      in_=tmp[:].rearrange("p (t d) -> p t d", d=D),
                      axis=mybir.AxisListType.X,
                      op=mybir.AluOpType.add,
                  )
                  if i == 0:
                      nc.vector.tensor_reduce(
                          out=red2[:, 1:2],
                          in_=sq[:],
                          axis=mybir.AxisListType.X,
                          op=mybir.AluOpType.add,
                      )
                  else:
                      sq1 = bpool.tile([P, 1], f32, name="sq1")
                      nc.vector.tensor_reduce(
                          out=sq1[:],
                          in_=sq[:],
                          axis=mybir.AxisListType.X,
                          op=mybir.AluOpType.add,
                      )
                      nc.vector.tensor_tensor(
                          out=red2[:, 1:2],
                          in0=red2[:, 1:2],
                          in1=sq1[:],
                          op=mybir.AluOpType.add,
                      )
              # partition reduce via ones matmul: out[2,1] = red2.T @ ones
              bp_ps = bppool.tile([2, 1], f32, space="PSUM")
              nc.tensor.matmul(
                  out=bp_ps[:], lhsT=red2[:], rhs=ones_sb[:], start=True, stop=True
              )
              sc = bpool.tile([2, 1], f32, name="sc")
              nc.vector.tensor_copy(out=sc[:], in_=bp_ps[:])
              nc.sync.dma_start(out=out_sc[:], in_=sc[:])

    nc.compile()
    return nc


_LAST_EXEC_NS = None
_LAST_RUN_SECONDS = None
_LAST_RES = None


def kernel(user_emb, item_emb, edge_vals, edge_src, edge_dst, users, pos, neg):
    global _LAST_EXEC_NS, _LAST_RUN_SECONDS, _LAST_RES
    import time as _time

    from concourse.bass_utils import run_bass_kernel_spmd

    x0, static, percore = preprocess(
        user_emb, item_emb, edge_vals, edge_src, edge_dst, users, pos, neg
    )
    nc = build_program(static)

    import ml_dtypes

    iota = np.broadcast_to(
        np.arange(2 * P, dtype=np.float32).astype(ml_dtypes.bfloat16), (P, 2 * P)
    ).copy()
    ones = np.ones((P, 1), dtype=np.float32)
    in_maps = []
    for c in range(CORES):
        pc = percore[c]
        in_maps.append(
            {
                "x0": x0,
                "x0bf": static["x0bf"],
                "x0_slab": pc["x0_slab"],
                "idx": pc["idx"],
                "val": pc["val"],
                "dst": pc["dst"],
                "iota": iota,
                "ones": ones,
                "u_idx": pc["u_idx"],
                "p_idx": pc["p_idx"],
                "n_idx": pc["n_idx"],
            }
        )

    _t0 = _time.time()
    res = run_bass_kernel_spmd(nc, in_maps, core_ids=list(range(CORES)))
    _LAST_RUN_SECONDS = _time.time() - _t0
    _LAST_EXEC_NS = res.exec_time_ns
    _LAST_RES = res
    loss = np.float32(0.0)
    reg_raw = np.float32(0.0)
    for c in range(CORES):
        sc = res.results[c]["out_sc"]
        loss += sc[0, 0]
        reg_raw += sc[1, 0]
    reg_loss = np.float32(0.5) * reg_raw / np.float32(BATCH)
    return np.float32(loss), np.float32(reg_loss)



# revision 24
# speedup vs baseline: 2.0429x; 1.7576x over previous
"""LightGCN (3-layer) + BPR loss on 8 Trainium2 NeuronCores.

Strategy (graph-parallel over edge destinations):
  - Remap nodes so core c owns a contiguous padded slab of 20480 node slots
    (160 windows x 128); sort edges by destination and shard by dst slab.
  - Per layer, per core: gather x[src] rows (256B) with dma_gather (int16
    local indices, node table split into 5 x 32768-row chunks), build a
    scaled one-hot S[e, dst_local] = val_e with one DVE tensor_scalar
    (iota, is_equal, mult), and segment-sum via TensorE matmuls
    accumulating in PSUM over a 16-window superblock. Flush adds into an
    SBUF-resident acc and writes the slab; AllGather replicates the new
    node table for the next layer's gathers.
  - BPR tail is data-parallel over the 4096 batch (512/core): indirect
    row gathers + DVE dot products + ScalarE softplus + a ones-matmul
    partition reduction. Host sums the 8 partial (loss, reg) pairs.
"""

import sys

sys.path.insert(0, "/opt/trn_rl_repo")

import numpy as np

P = 128
D = 64
CORES = 8
N_USERS = 100000
N_ITEMS = 50000
N = N_USERS + N_ITEMS  # 150000
SLAB_REAL = N // CORES  # 18750
WPC = 160  # windows per core (147 real + 13 dead, for 10 uniform superblocks)
SLABP = WPC * P  # 20480 padded node slots per core
NP_TOTAL = CORES * SLABP  # 163840
CHUNK = 32768  # dma_gather int16 index reach
NCHUNK = NP_TOTAL // CHUNK  # 5
SBW = 16  # windows per superblock
NSB = WPC // SBW  # 10
BATCH = 4096
BSH = BATCH // CORES  # 512 batch rows per core
BT = BSH // P  # 4 batch tiles per core


def _remap(n):
    """global node id -> padded id (core-contiguous slabs)"""
    return (n // SLAB_REAL) * SLABP + (n % SLAB_REAL)


def preprocess(user_emb, item_emb, edge_vals, edge_src, edge_dst, users, pos, neg):
    """Host-side: build the padded node table, per-core edge streams, static
    tile maps shared by all cores, and BPR index tiles."""
    src_m = _remap(edge_src.astype(np.int64))
    dst_m = _remap(edge_dst.astype(np.int64))
    val = edge_vals.astype(np.float32)

    x0 = np.zeros((NP_TOTAL, D), dtype=np.float32)
    emb = np.concatenate([user_emb, item_emb], axis=0).astype(np.float32)
    x0[_remap(np.arange(N))] = emb
    # doubled-row bf16 table: row i = [x[i], zeros]; 256B rows keep dma_gather
    # legal while all compute runs bf16. cols 64:128 are never read.
    import ml_dtypes

    x0bf = np.zeros((NP_TOTAL, 2 * D), dtype=ml_dtypes.bfloat16)
    x0bf[:, :D] = x0.astype(ml_dtypes.bfloat16)

    core = dst_m // SLABP
    dst_local = dst_m - core * SLABP
    win = dst_local >> 7  # window within core
    chunk = src_m // CHUNK
    sb = win // SBW
    wr = win - sb * SBW  # window within superblock
    wkey = np.where(chunk % 2 == 0, wr, SBW - 1 - wr)  # serpentine

    # per (core, window, chunk) counts -> static quotas (max over cores)
    flat = (core * WPC + win) * NCHUNK + chunk
    counts = np.bincount(flat, minlength=CORES * WPC * NCHUNK).reshape(
        CORES, WPC, NCHUNK
    )
    Q = counts.max(axis=0)  # [WPC, NCHUNK]

    # static slot layout: superblock -> chunk -> serpentine windows
    # group sizes (pre-pad) per (sb, chunk)
    grp_sizes = np.zeros((NSB, NCHUNK), dtype=np.int64)
    for s in range(NSB):
        for c in range(NCHUNK):
            grp_sizes[s, c] = Q[s * SBW : (s + 1) * SBW, c].sum()
    grp_pad = ((grp_sizes + P - 1) // P) * P  # 128-aligned groups
    sb_sizes = grp_pad.sum(axis=1)  # slots per superblock
    sb_starts = np.concatenate([[0], np.cumsum(sb_sizes)])
    TOT = int(sb_starts[-1])
    NTILES = TOT // P

    # run starts per (window, chunk) in slot space + per-slot window map
    run_start = np.zeros((WPC, NCHUNK), dtype=np.int64)
    W_slot = np.zeros(TOT, dtype=np.int32)
    grp_start = np.zeros((NSB, NCHUNK), dtype=np.int64)
    for s in range(NSB):
        off = sb_starts[s]
        for c in range(NCHUNK):
            grp_start[s, c] = off
            ws = range(s * SBW, (s + 1) * SBW)
            order = list(ws) if c % 2 == 0 else list(ws)[::-1]
            last_w = order[0]
            for w in order:
                run_start[w, c] = off
                W_slot[off : off + Q[w, c]] = w
                if Q[w, c] > 0:
                    last_w = w
                off += Q[w, c]
            # group-end pad slots: last window that actually received slots
            pad_end = grp_start[s, c] + grp_pad[s, c]
            if off < pad_end:
                W_slot[off:pad_end] = last_w
            off = pad_end

    # tile maps (static, same all cores)
    tw = W_slot.reshape(NTILES, P)
    tile_minw = tw.min(axis=1)
    tile_maxw = tw.max(axis=1)
    assert (tile_maxw - tile_minw <= 1).all(), "tile spans >2 windows"
    # which tile is each window's first/last segment (slot order)
    first_tile = np.full(WPC, -1, dtype=np.int64)
    last_tile = np.full(WPC, -1, dtype=np.int64)
    for t in range(NTILES):
        for w in range(tile_minw[t], tile_maxw[t] + 1):
            if first_tile[w] < 0:
                first_tile[w] = t
            last_tile[w] = t

    # fill per-core streams
    dst_rel_default = (W_slot.astype(np.int64) * P) - tile_minw.repeat(P) * P
    idx_all = np.zeros((CORES, TOT), dtype=np.int16)  # chunk-local src idx
    val_all = np.zeros((CORES, TOT), dtype=np.float32)
    dstr_all = np.broadcast_to(
        dst_rel_default.astype(np.float32), (CORES, TOT)
    ).copy()

    # slot assignment: order edges by (core, run), cumcount within run
    run_id = (core * WPC + win) * NCHUNK + chunk
    order = np.lexsort((src_m, run_id))
    rid_s = run_id[order]
    # position within run (rid_s is sorted)
    starts = np.concatenate([[0], np.flatnonzero(rid_s[1:] != rid_s[:-1]) + 1])
    lens = np.diff(np.concatenate([starts, [len(rid_s)]]))
    run_pos = np.arange(len(rid_s)) - np.repeat(starts, lens)
    slot = run_start[win[order], chunk[order]] + run_pos
    c_o = core[order]
    idx_all[c_o, slot] = (src_m[order] - chunk[order] * CHUNK).astype(np.int16)
    val_all[c_o, slot] = val[order]
    dstr = dst_local[order] - tile_minw[slot // P].astype(np.int64) * P
    dstr_all[c_o, slot] = dstr.astype(np.float32)

    # wrap idxs per gather group: [TOT] -> [128, TOT//16] (16-wrap, replicated)
    idx_wrapped = np.zeros((CORES, P, TOT // 16), dtype=np.int16)
    for s in range(NSB):
        for c in range(NCHUNK):
            g0, g1 = grp_start[s, c], grp_start[s, c] + grp_pad[s, c]
            blk = idx_all[:, g0:g1].reshape(CORES, -1, 16).transpose(0, 2, 1)
            idx_wrapped[:, :16, g0 // 16 : g1 // 16] = blk
    idx_wrapped[:, 16:, :] = np.tile(idx_wrapped[:, :16, :], (1, 7, 1))

    # [TOT] -> [128, NTILES] tile-major for dst/val (f32: tensor_scalar
    # scalar operands must be f32)
    val_t = val_all.reshape(CORES, NTILES, P).transpose(0, 2, 1).copy()
    dst_t = dstr_all.reshape(CORES, NTILES, P).transpose(0, 2, 1).copy()

    # per-core x0 slab (for acc init)
    x0_slab = x0.reshape(CORES, SLABP, D)

    # BPR per-core index tiles [128, BT] int32 (padded-id row offsets)
    def btile(ids):
        return ids.reshape(BT, P).T.astype(np.int32).copy()

    u_g = _remap(users.astype(np.int64))
    p_g = _remap(N_USERS + pos.astype(np.int64))
    n_g = _remap(N_USERS + neg.astype(np.int64))
    bpr = np.stack([u_g, p_g, n_g]).reshape(3, CORES, BSH)  # [3, CORES, 512]

    static = dict(
        TOT=TOT,
        NTILES=NTILES,
        x0bf=x0bf,
        grp_start=grp_start,
        grp_pad=grp_pad,
        sb_starts=sb_starts,
        tile_minw=tile_minw,
        tile_maxw=tile_maxw,
        first_tile=first_tile,
        last_tile=last_tile,
        W_slot=W_slot,
    )
    percore = []
    for c in range(CORES):
        percore.append(
            dict(
                idx=idx_wrapped[c],
                val=val_t[c],
                dst=dst_t[c],
                x0_slab=x0_slab[c].copy(),
                u_idx=btile(bpr[0, c]),
                p_idx=btile(bpr[1, c]),
                n_idx=btile(bpr[2, c]),
            )
        )
    return x0, static, percore


def numpy_mirror(x0, static, percore):
    """Debug: simulate the device computation with numpy. Returns the new
    full node table after one propagation layer (all cores)."""
    TOT, NTILES = static["TOT"], static["NTILES"]
    tile_minw = static["tile_minw"]
    out = np.zeros((NP_TOTAL, D), dtype=np.float32)
    for c in range(CORES):
        pc = percore[c]
        # unwrap idx
        idx = np.zeros(TOT, dtype=np.int64)
        for s in range(NSB):
            for ch in range(NCHUNK):
                g0 = static["grp_start"][s, ch]
                g1 = g0 + static["grp_pad"][s, ch]
                blk = pc["idx"][:16, g0 // 16 : g1 // 16]
                idx[g0:g1] = blk.T.reshape(-1).astype(np.int64) + ch * CHUNK
        gathered = x0[idx]  # [TOT, D]
        val = pc["val"].T.reshape(-1)
        dstr = pc["dst"].T.reshape(-1).astype(np.int64)
        node = dstr + tile_minw.repeat(P) * P + c * SLABP
        np.add.at(out, node, gathered * val[:, None])
    return out


def build_program(static, nsb_limit=NSB, nlayers=3, do_ag=True, do_bpr=True,
                  do_gather=True, do_s=True, do_mm=True, do_flush=True,
                  do_idxload=True):
    import concourse.bacc as bacc
    import concourse.bass as bass
    import concourse.mybir as mybir
    import concourse.tile as tile

    TOT, NTILES = static["TOT"], static["NTILES"]
    grp_start, grp_pad = static["grp_start"], static["grp_pad"]
    sb_starts = static["sb_starts"]
    tile_minw, tile_maxw = static["tile_minw"], static["tile_maxw"]
    first_tile, last_tile = static["first_tile"], static["last_tile"]

    f32 = mybir.dt.float32
    bf16 = mybir.dt.bfloat16
    nc = bacc.Bacc(
        "TRN2",
        target_bir_lowering=False,
        debug=False,
        num_devices=CORES,
        num_swdge_queues=4,
    )

    x0_ext = nc.dram_tensor("x0", [NP_TOTAL, D], f32, kind="ExternalInput")
    x0bf_ext = nc.dram_tensor("x0bf", [NP_TOTAL, 2 * D], bf16, kind="ExternalInput")
    x0_slab = nc.dram_tensor("x0_slab", [SLABP, D], f32, kind="ExternalInput")
    idx_in = nc.dram_tensor("idx", [P, TOT // 16], mybir.dt.int16, kind="ExternalInput")
    val_in = nc.dram_tensor("val", [P, NTILES], f32, kind="ExternalInput")
    dst_in = nc.dram_tensor("dst", [P, NTILES], f32, kind="ExternalInput")
    iota_in = nc.dram_tensor("iota", [P, 2 * P], bf16, kind="ExternalInput")
    ones_in = nc.dram_tensor("ones", [P, 1], f32, kind="ExternalInput")
    u_in = nc.dram_tensor("u_idx", [P, BT], mybir.dt.int32, kind="ExternalInput")
    p_in = nc.dram_tensor("p_idx", [P, BT], mybir.dt.int32, kind="ExternalInput")
    n_in = nc.dram_tensor("n_idx", [P, BT], mybir.dt.int32, kind="ExternalInput")
    out_sc = nc.dram_tensor("out_sc", [2, 1], f32, kind="ExternalOutput")

    with tile.TileContext(nc) as tc:
        with (
            tc.tile_pool(name="const", bufs=1) as cpool,
            tc.tile_pool(name="acc", bufs=1) as apool,
            tc.tile_pool(name="stream", bufs=2) as stpool,
            tc.tile_pool(name="idxp", bufs=8) as idxpool,
            tc.tile_pool(name="gb", bufs=4) as gpool,
            tc.tile_pool(name="s", bufs=12) as spool,
            tc.tile_pool(name="fl", bufs=2) as fpool,
            tc.tile_pool(name="psum", bufs=2, space="PSUM") as ppool,
            tc.tile_pool(name="bsum", bufs=1, space="PSUM") as bppool,
            tc.tile_pool(name="bpr", bufs=1) as bpool,
            tc.tile_pool(name="dram", bufs=1, space="DRAM") as dpool,
        ):
            iota_sb = cpool.tile([P, 2 * P], bf16)
            nc.sync.dma_start(out=iota_sb[:], in_=iota_in[:])
            # PSUM-resident f32 iota: tensor_scalar reading in0 from PSUM runs
            # in 1-port mode, avoiding the SWDGE/DVE SBUF port mutex that
            # stalls DVE ops for the whole length of a gather's desc emission
            iota_ps = bppool.tile([P, 512], f32, space="PSUM", name="iota_ps")
            nc.vector.tensor_copy(out=iota_ps[:, : 2 * P], in_=iota_sb[:])
            ones_sb = cpool.tile([P, 1], f32)
            nc.sync.dma_start(out=ones_sb[:], in_=ones_in[:])

            # SBUF-resident accumulator [128, WPC*D], window w at cols w*D
            acc_sb = apool.tile([P, WPC * D], f32)
            nc.sync.dma_start(
                out=acc_sb[:].rearrange("p (w d) -> p w d", d=D),
                in_=x0_slab[:].rearrange("(w p) d -> p w d", p=P),
            )

            # DRAM internals (node tables are doubled-row bf16; cols D:2D unread)
            slab_dram = [
                dpool.tile([SLABP, 2 * D], bf16, name=f"slab{l}") for l in range(3)
            ]
            xg = [dpool.tile([NP_TOTAL, 2 * D], bf16, name=f"xg{l}") for l in range(2)]
            acc_slab_dram = dpool.tile([SLABP, D], f32)
            acc_full = dpool.tile([NP_TOTAL, D], f32)

            gsrc = [x0bf_ext, xg[0]] + [xg[1]] * max(1, nlayers - 2)

            max_grp = int(grp_pad.max())
            gq = 0
            idx_fixed = None
            if not do_idxload:
                idx_fixed = cpool.tile([P, max_grp // 16], mybir.dt.int16)
                nc.sync.dma_start(
                    out=idx_fixed[:], in_=idx_in[:, : max_grp // 16]
                )
            for layer in range(nlayers):
                src_t = gsrc[layer]
                for s in range(nsb_limit):
                    t0 = int(sb_starts[s]) // P
                    t1 = int(sb_starts[s + 1]) // P
                    # stream tiles for this superblock
                    ntile_s = t1 - t0
                    val_sb = stpool.tile([P, ntile_s], f32, tag="val")
                    nc.sync.dma_start(out=val_sb[:], in_=val_in[:, t0:t1])
                    dst_sb = stpool.tile([P, ntile_s], f32, tag="dst")
                    nc.sync.dma_start(out=dst_sb[:], in_=dst_in[:, t0:t1])

                    psum = ppool.tile([P, SBW * D], f32, space="PSUM")
                    for ch in range(NCHUNK):
                        g0 = int(grp_start[s, ch])
                        gl = int(grp_pad[s, ch])
                        if gl == 0:
                            continue
                        if do_idxload:
                            idx_sb = idxpool.tile(
                                [P, max_grp // 16], mybir.dt.int16, tag="idx"
                            )
                            nc.sync.dma_start(
                                out=idx_sb[:, : gl // 16],
                                in_=idx_in[:, g0 // 16 : (g0 + gl) // 16],
                            )
                        else:
                            idx_sb = idx_fixed
                        gbuf = gpool.tile([P, (max_grp // P) * 2 * D], bf16, tag="gbuf")
                        if do_gather:
                          nc.gpsimd.dma_gather(
                            gbuf[:, : (gl // P) * 2 * D].rearrange(
                                "p (t d) -> p t d", d=2 * D
                            ),
                            src_t[ch * CHUNK : (ch + 1) * CHUNK, :],
                            idx_sb[:, : gl // 16],
                            gl,
                            gl,
                            2 * D,
                            single_packet=False,
                            queue_num=gq % 4,
                          )
                        gq += 1
                        for tt in range(g0 // P, (g0 + gl) // P):
                            trel = tt - t0
                            gt = tt - g0 // P
                            minw, maxw = int(tile_minw[tt]), int(tile_maxw[tt])
                            nwin = maxw - minw + 1
                            s_t = spool.tile([P, 2 * P], bf16, tag="s_t")
                            if do_s:
                              nc.vector.tensor_scalar(
                                out=s_t[:, : nwin * P],
                                in0=iota_ps[:, : nwin * P],
                                scalar1=dst_sb[:, trel : trel + 1],
                                scalar2=val_sb[:, trel : trel + 1],
                                op0=mybir.AluOpType.is_equal,
                                op1=mybir.AluOpType.mult,
                              )
                            for k in range(nwin):
                                w = minw + k
                                wr = w - s * SBW
                                if do_mm:
                                    nc.tensor.matmul(
                                        out=psum[:, wr * D : (wr + 1) * D],
                                        lhsT=s_t[:, k * P : (k + 1) * P],
                                        rhs=gbuf[:, gt * 2 * D : gt * 2 * D + D],
                                        start=(first_tile[w] == tt),
                                        stop=(last_tile[w] == tt),
                                    )

                    # flush superblock: ACT casts psum -> bf16 slab tile, DVE
                    # adds psum into the f32 acc, slab x-half written to DRAM
                    if not do_flush:
                        continue
                    flush = fpool.tile([P, SBW * D], bf16, tag="flush")
                    nc.scalar.copy(out=flush[:], in_=psum[:])
                    nc.vector.tensor_tensor(
                        out=acc_sb[:, s * SBW * D : (s + 1) * SBW * D],
                        in0=acc_sb[:, s * SBW * D : (s + 1) * SBW * D],
                        in1=psum[:],
                        op=mybir.AluOpType.add,
                    )
                    nc.sync.dma_start(
                        out=slab_dram[min(layer, 2)][
                            s * SBW * P : (s + 1) * SBW * P, :D
                        ].rearrange("(w p) d -> p w d", p=P),
                        in_=flush[:].rearrange("p (w d) -> p w d", d=D),
                    )

                if layer < 2 and do_ag:
                    nc.gpsimd.collective_compute(
                        "AllGather",
                        mybir.AluOpType.bypass,
                        replica_groups=[list(range(CORES))],
                        ins=[slab_dram[layer].opt()],
                        outs=[xg[layer].opt()],
                    )

            # acc -> DRAM slab -> AllGather
            if do_ag:
              nc.sync.dma_start(
                out=acc_slab_dram[:].rearrange("(w p) d -> p w d", p=P),
                in_=acc_sb[:].rearrange("p (w d) -> p w d", d=D),
              )
              nc.gpsimd.collective_compute(
                "AllGather",
                mybir.AluOpType.bypass,
                replica_groups=[list(range(CORES))],
                ins=[acc_slab_dram.opt()],
                outs=[acc_full.opt()],
              )

            # ---- BPR tail ----
            if not do_bpr:
                zt = bpool.tile([2, 1], f32, name='zt')
                nc.vector.memset(zt[:], 0.0)
                nc.sync.dma_start(out=out_sc[:], in_=zt[:])
            else:
              bidx = {"u": u_in, "p": p_in, "n": n_in}
              bsb = {}
              for k, t_in in bidx.items():
                  tl = bpool.tile([P, BT], mybir.dt.int32, name=f"bi_{k}")
                  nc.sync.dma_start(out=tl[:], in_=t_in[:])
                  bsb[k] = tl

              def gather_rows(table, idx_tile, name):
                  dst = bpool.tile([P, BT * D], f32, name=f"g_{name}")
                  for j in range(BT):
                      nc.gpsimd.indirect_dma_start(
                          out=dst[:, j * D : (j + 1) * D],
                          out_offset=None,
                          in_=table[:],
                          in_offset=bass.IndirectOffsetOnAxis(
                              ap=idx_tile[:, j : j + 1], axis=0
                          ),
                      )
                  return dst

              gu = gather_rows(acc_full, bsb["u"], "u")
              gp = gather_rows(acc_full, bsb["p"], "p")
              gn = gather_rows(acc_full, bsb["n"], "n")
              g0u = gather_rows(x0_ext, bsb["u"], "u0")
              g0p = gather_rows(x0_ext, bsb["p"], "p0")
              g0n = gather_rows(x0_ext, bsb["n"], "n0")

              # lightgcn output = acc / 4
              # scores: sum over D of (gu/4)*(gp/4) = dot(gu,gp)/16
              tmp = bpool.tile([P, BT * D], f32, name="tmp")
              ps = bpool.tile([P, BT], f32, name="ps")
              ns_ = bpool.tile([P, BT], f32, name="ns")
              nc.vector.tensor_tensor(
                  out=tmp[:], in0=gu[:], in1=gp[:], op=mybir.AluOpType.mult
              )
              nc.vector.tensor_reduce(
                  out=ps[:],
                  in_=tmp[:].rearrange("p (t d) -> p t d", d=D),
                  axis=mybir.AxisListType.X,
                  op=mybir.AluOpType.add,
              )
              nc.vector.tensor_tensor(
                  out=tmp[:], in0=gu[:], in1=gn[:], op=mybir.AluOpType.mult
              )
              nc.vector.tensor_reduce(
                  out=ns_[:],
                  in_=tmp[:].rearrange("p (t d) -> p t d", d=D),
                  axis=mybir.AxisListType.X,
                  op=mybir.AluOpType.add,
              )
              # diff = (ns - ps)/16 ; softplus ; sum over batch tiles
              diff = bpool.tile([P, BT], f32, name="diff")
              nc.vector.tensor_tensor(
                  out=diff[:], in0=ns_[:], in1=ps[:], op=mybir.AluOpType.subtract
              )
              # softplus(diff/16) = ln(1 + exp(diff/16)); scores are tiny so
              # exp cannot overflow
              sp = bpool.tile([P, BT], f32, name="sp")
              nc.scalar.activation(
                  out=sp[:],
                  in_=diff[:],
                  func=mybir.ActivationFunctionType.Exp,
                  scale=1.0 / 16.0,
              )
              nc.vector.tensor_scalar(
                  out=sp[:],
                  in0=sp[:],
                  scalar1=1.0,
                  scalar2=None,
                  op0=mybir.AluOpType.add,
              )
              nc.scalar.activation(
                  out=sp[:], in_=sp[:], func=mybir.ActivationFunctionType.Ln
              )
              # reg: sum of squares of u0,p0,n0
              sq = bpool.tile([P, BT], f32, name="sq")
              red2 = bpool.tile([P, 2], f32, name="red2")
              nc.vector.tensor_reduce(
                  out=red2[:, 0:1],
                  in_=sp[:],
                  axis=mybir.AxisListType.X,
                  op=mybir.AluOpType.add,
              )
              for i, g in enumerate([g0u, g0p, g0n]):
                  nc.vector.tensor_tensor(
                      out=tmp[:], in0=g[:], in1=g[:], op=mybir.AluOpType.mult
                  )
                  nc.vector.tensor_reduce(
                      out=sq[:],
                      in_=tmp[:].rearrange("p (t d) -> p t d", d=D),
                      axis=mybir.AxisListType.X,
                      op=mybir.AluOpType.add,
                  )
                  if i == 0:
                      nc.vector.tensor_reduce(
                          out=red2[:, 1:2],
                          in_=sq[:],
                          axis=mybir.AxisListType.X,
                          op=mybir.AluOpType.add,
                      )
                  else:
                      sq1 = bpool.tile([P, 1], f32, name="sq1")
                      nc.vector.tensor_reduce(
                          out=sq1[:],
                          in_=sq[:],
                          axis=mybir.AxisListType.X,
                          op=mybir.AluOpType.add,
                      )
                      nc.vector.tensor_tensor(
                          out=red2[:, 1:2],
                          in0=red2[:, 1:2],
                          in1=sq1[:],
                          op=mybir.AluOpType.add,
                      )
              # partition reduce via ones matmul: out[2,1] = red2.T @ ones
              bp_ps = bppool.tile([2, 1], f32, space="PSUM")
              nc.tensor.matmul(
                  out=bp_ps[:], lhsT=red2[:], rhs=ones_sb[:], start=True, stop=True
              )
              sc = bpool.tile([2, 1], f32, name="sc")
              nc.vector.tensor_copy(out=sc[:], in_=bp_ps[:])
              nc.sync.dma_start(out=out_sc[:], in_=sc[:])

    nc.compile()
    return nc


_LAST_EXEC_NS = None
_LAST_RUN_SECONDS = None
_LAST_RES = None


def kernel(user_emb, item_emb, edge_vals, edge_src, edge_dst, users, pos, neg):
    global _LAST_EXEC_NS, _LAST_RUN_SECONDS, _LAST_RES
    import time as _time

    from concourse.bass_utils import run_bass_kernel_spmd

    x0, static, percore = preprocess(
        user_emb, item_emb, edge_vals, edge_src, edge_dst, users, pos, neg
    )
    nc = build_program(static)

    import ml_dtypes

    iota = np.broadcast_to(
        np.arange(2 * P, dtype=np.float32).astype(ml_dtypes.bfloat16), (P, 2 * P)
    ).copy()
    ones = np.ones((P, 1), dtype=np.float32)
    in_maps = []
    for c in range(CORES):
        pc = percore[c]
        in_maps.append(
            {
                "x0": x0,
                "x0bf": static["x0bf"],
                "x0_slab": pc["x0_slab"],
                "idx": pc["idx"],
                "val": pc["val"],
                "dst": pc["dst"],
                "iota": iota,
                "ones": ones,
                "u_idx": pc["u_idx"],
                "p_idx": pc["p_idx"],
                "n_idx": pc["n_idx"],
            }
        )

    _t0 = _time.time()
    res = run_bass_kernel_spmd(nc, in_maps, core_ids=list(range(CORES)))
    _LAST_RUN_SECONDS = _time.time() - _t0
    _LAST_EXEC_NS = res.exec_time_ns
    _LAST_RES = res
    loss = np.float32(0.0)
    reg_raw = np.float32(0.0)
    for c in range(CORES):
        sc = res.results[c]["out_sc"]
        loss += sc[0, 0]
        reg_raw += sc[1, 0]
    reg_loss = np.float32(0.5) * reg_raw / np.float32(BATCH)
    return np.float32(loss), np.float32(reg_loss)

